# revision 44
# baseline (speedup 1.0000x reference)
"""Causal GQA self-attention (B=2, T=2048, C=2048, 16 heads / 4 KV groups,
head_size=128, RoPE) on 8 Trainium2 NeuronCores.

Sharding: tensor-parallel over the 4 KV groups x data-parallel over the 2
batch elements -> 8 cores, core = b*4 + g. Each core computes its group's
QKV projection, RoPE, causal SDPA for the group's 4 query heads, and the
partial output projection (w_proj input-dim shard). The proj partials are
reduced on the host (equivalent of the post-proj all-reduce).

All matmuls run in bf16 with fp32 PSUM accumulation. Inputs are transposed
and cast to bf16 on the host so every DMA is a contiguous, layout-perfect
load (contraction dims land on SBUF partitions).

Schedule notes (v7):
- Warmup junk matmuls (junk memset on DVE, first thing) + a dummy Exp run
  while the first DMAs land: PE p-state ramp + ACT exp-table load happen
  during dead startup time.
- w_attn is split host-side into wkv [C,256] and wq [C,512] chunk
  streams; the k+v projection runs 7 accumulation chains interleaved
  ci-outer so the PE consumes each (x, wkv) chunk pair at the DMA
  delivery rate. cos/sin land before the RoPE drains; wq streams in
  while the v-t4=0 chain and the first q chain run.
- Separate kvps/qps PSUM pool lifetimes: pool-level released-zone deps
  mean a new pool waits for ALL drains of any overlapped released pool,
  so attention's scores/y tiles are placed on banks whose pools released
  tens of microseconds earlier; the ones-matmul/proj pools tolerate the
  q-phase drain tail.
- v^T -> v t-major PE transposes ride one-per-q-chain (tb ascending);
  copies drain on ACT.
- The causal mask for diagonal blocks is preloaded into PSUM with a
  mask-base matmul and the scores matmul accumulates on top in two
  column pieces - no DVE hop in the scores->exp chain.
- Row sums: full-width exp strips accumulate sequentially in groups of 4
  (adds 1,2 on DVE, final on GpSimd), one ones-matmul per group issued 4
  blocks later; diagonal strips go straight to the ones-matmul.
- The partial output projection of quarter q is interleaved matmul-by-
  matmul into quarter q-1's attention loop: the ACT exp stream (~750ns
  per 512-wide strip) otherwise paces the PE (~650ns per j-block), so
  giving the PE extra independent work removes the exp-pacing bubbles.
  PSUM->SBUF copies ride DVE in bf16; bf16 output DMA (host upcasts and
  reduces in fp32).
"""

import sys
import math

for _p in ("/opt/trn_rl_repo", "/root/.axon_site/_ro/trn_rl_repo"):
    if _p not in sys.path:
        sys.path.insert(0, _p)

import numpy as np
import ml_dtypes

import concourse.bass as bass  # noqa: F401  (registers engine classes)
import concourse.bacc as bacc
import concourse.tile as tile
from concourse import mybir
from concourse.bass_utils import run_bass_kernel_spmd
from concourse.masks import make_identity
from contextlib import ExitStack

BF16 = ml_dtypes.bfloat16
P = 128
T = 2048
C = 2048
NT = T // P        # 16 t-blocks
NCC = C // P       # 16 contraction chunks
NQ = 4             # query heads per core
FQ = NQ * P        # 512 (q rows per group)
FKV = 2 * P        # 256 (k+v rows per group)
FY = NQ * P        # 512
SCALE = 1.0 / math.sqrt(P)
NEG = -1.0e30

dt = mybir.dt
AF = mybir.ActivationFunctionType
ALU = mybir.AluOpType

TRACE = False
_CACHE = {}


def _build():
    nc = bacc.Bacc("TRN2", target_bir_lowering=False, debug=False, num_devices=8)
    xT_d = nc.dram_tensor("xT", [C, T], dt.bfloat16, kind="ExternalInput").ap()
    wqT_d = nc.dram_tensor("wqT", [C, FQ], dt.bfloat16, kind="ExternalInput").ap()
    wkvT_d = nc.dram_tensor("wkvT", [C, FKV], dt.bfloat16, kind="ExternalInput").ap()
    wpT_d = nc.dram_tensor("wpT", [FY, T], dt.bfloat16, kind="ExternalInput").ap()
    cosT_d = nc.dram_tensor("cosT", [P, T], dt.bfloat16, kind="ExternalInput").ap()
    sinS_d = nc.dram_tensor("sinS", [P, T], dt.bfloat16, kind="ExternalInput").ap()
    out_d = nc.dram_tensor("out", [T, C], dt.bfloat16, kind="ExternalOutput").ap()

    with tile.TileContext(nc) as tc, ExitStack() as ctx:
        const = ctx.enter_context(tc.tile_pool(name="const", bufs=1))
        # junk tile memset FIRST (on DVE, which idles at startup) so PE
        # warmup can begin as soon as the engine queues come up.
        junk = const.tile([P, 512], dt.bfloat16, tag="junk", name="junk")
        nc.vector.memset(junk, 0.125)
        junk_exp = const.tile([P, 1], dt.float32, tag="jexp", name="junk_exp")
        identity = const.tile([P, P], dt.bfloat16, tag="id", name="identity")
        make_identity(nc, identity)
        ones_bf = const.tile([P, P], dt.bfloat16, tag="ones", name="ones_bf")
        nc.gpsimd.memset(ones_bf, 1.0)
        # transposed causal mask for the diagonal 128x128 block: loaded into
        # PSUM via matmul (lhsT=mskT, rhs=identity) so the PSUM base is
        # mskT.T: element (p=j, f=i) = 0 where i - j >= 0, else -1e30.
        mskT = const.tile([P, P], dt.bfloat16, tag="mskT", name="mskT")
        nc.gpsimd.memset(mskT, 0.0)
        nc.gpsimd.affine_select(
            out=mskT, in_=mskT, compare_op=ALU.is_ge, fill=NEG,
            base=0, pattern=[[-1, P]], channel_multiplier=1,
        )

        trig = ctx.enter_context(tc.tile_pool(name="trig", bufs=1))
        cosT = trig.tile([P, T], dt.bfloat16, tag="cos", name="cosT")
        sinS = trig.tile([P, T], dt.bfloat16, tag="sin", name="sinS")

        persist = ctx.enter_context(tc.tile_pool(name="persist", bufs=1))
        qrot = [persist.tile([P, T], dt.bfloat16, tag=f"q{h}", name=f"q{h}") for h in range(NQ)]
        krot = persist.tile([P, T], dt.bfloat16, tag="k", name="krot")
        vraw = persist.tile([P, T], dt.bfloat16, tag="vr", name="vraw")   # v^T (d-major)
        vt = persist.tile([P, T], dt.bfloat16, tag="vt", name="vt")       # v t-major blocks
        y_sb = [persist.tile([P, T], dt.bfloat16, tag=f"y{h}", name=f"ysb{h}") for h in range(NQ)]
        wp_t = [persist.tile([P, T], dt.bfloat16, tag=f"wp{j}", name=f"wp{j}") for j in range(NQ)]

        # DMA order: (x, wkv) chunk pairs stream first and pace the k+v
        # phase; cos/sin slot in before the RoPE drains need them; the wq
        # chunks follow (consumed from the first q chain onward), then wp.
        xw_pool = ctx.enter_context(tc.tile_pool(name="xw", bufs=1))
        xt, wkv, wq = [], [], []
        for ci in range(NCC):
            xt.append(xw_pool.tile([P, T], dt.bfloat16, tag=f"x{ci}", name=f"xt{ci}"))
            wkv.append(xw_pool.tile([P, FKV], dt.bfloat16, tag=f"wkv{ci}", name=f"wkv{ci}"))
            wq.append(xw_pool.tile([P, FQ], dt.bfloat16, tag=f"wq{ci}", name=f"wq{ci}"))

        def load_chunk(ci):
            nc.sync.dma_start(xt[ci], xT_d[ci * P:(ci + 1) * P, :])
            nc.sync.dma_start(wkv[ci], wkvT_d[ci * P:(ci + 1) * P, :])

        def load_wq(ci):
            nc.sync.dma_start(wq[ci], wqT_d[ci * P:(ci + 1) * P, :])

        # (x, wkv) pairs pace the k+v phase; cos/sin slot in before the
        # RoPE drains; wq follows the chunk stream (the v-t4=0 chain pads
        # the boundary, and the first q chain consumes wq gradually).
        for ci in range(0, 9):
            load_chunk(ci)
        nc.sync.dma_start(cosT, cosT_d)
        nc.sync.dma_start(sinS, sinS_d)
        for ci in range(9, NCC):
            load_chunk(ci)
        for ci in range(NCC):
            load_wq(ci)
        for j in range(NQ):
            nc.sync.dma_start(wp_t[j], wpT_d[j * P:(j + 1) * P, :])

        # Phase-2 SBUF pools allocated BEFORE rtmp (lower addresses) so the
        # attention phase never inherits rtmp's released-zone drain deps.
        strip_pool = ctx.enter_context(tc.tile_pool(name="strip", bufs=8))
        ssb_pool = ctx.enter_context(tc.tile_pool(name="ssb", bufs=3))
        ostage = ctx.enter_context(tc.tile_pool(name="ostage", bufs=4))
        gacc_pool = ctx.enter_context(tc.tile_pool(name="gacc", bufs=2))

        # ---------------- Phase 0: warmup ----------------------------------
        # PE p-state ramps to full clock after ~3us of continuous work; burn
        # the DMA-startup dead time on junk matmuls. Also trigger the ACT
        # exp table load now (1.3us) instead of at the first real softmax.
        nc.scalar.activation(junk_exp, junk[:, 0:1], AF.Exp, scale=1.0)
        with tc.tile_pool(name="warmps", bufs=2, space="PSUM") as warmps:
            for _ in range(9):
                wm_ps = warmps.tile([P, 512], dt.float32, tag="warm", name="warm_ps")
                nc.tensor.matmul(wm_ps, lhsT=junk[:, 0:P], rhs=junk,
                                 start=True, stop=True)

        # ---------------- Phase 1a: k/v projection, fused RoPE on k --------
        rtmp = ctx.enter_context(tc.tile_pool(name="rtmp", bufs=4))

        def rope_drain(dest, t4, ps, rot_eng=None):
            # RoPE (rotate-halves) in fp32, write bf16. The k drains pass
            # rot_eng=nc.gpsimd: all 4 kv chains stop only after the last
            # input chunk lands, so their rotate-muls would otherwise queue
            # ahead of the q-phase ropes on DVE and stall qps recycling.
            rot = rot_eng or nc.vector
            st = slice(t4 * 512, (t4 + 1) * 512)
            t1 = rtmp.tile([P, 512], dt.float32, tag="r1", name="ropet1")
            nc.vector.tensor_mul(t1, ps, cosT[:, st])
            t2 = rtmp.tile([P, 512], dt.float32, tag="r2", name="ropet2")
            rot.tensor_mul(t2[0:64, :], ps[64:128, :], sinS[0:64, st])
            rot.tensor_mul(t2[64:128, :], ps[0:64, :], sinS[64:128, st])
            nc.gpsimd.tensor_add(dest[:, st], t1, t2)

        with tc.tile_pool(name="kvps", bufs=7, space="PSUM") as kvps:
            # 7 accumulation chains interleaved ci-outer (k: t4=3..0,
            # v: t4=3..1) so the PE consumes chunks at the DMA rate;
            # v t4=0 follows as a single chain while the k drains run.
            kv_chains = [(0, 3), (0, 2), (0, 1), (0, 0), (1, 3), (1, 2), (1, 1)]
            pss = {c: kvps.tile([P, 512], dt.float32, tag="qkv", name="kvps_t")
                   for c in kv_chains}
            for ci in range(NCC):
                for fkv, t4 in kv_chains:
                    nc.tensor.matmul(
                        pss[(fkv, t4)],
                        lhsT=wkv[ci][:, fkv * P:(fkv + 1) * P],
                        rhs=xt[ci][:, t4 * 512:(t4 + 1) * 512],
                        start=(ci == 0), stop=(ci == NCC - 1),
                    )
            for t4 in (3, 2, 1, 0):
                rope_drain(krot, t4, pss[(0, t4)])
                if t4 > 0:
                    nc.scalar.copy(vraw[:, t4 * 512:(t4 + 1) * 512], pss[(1, t4)])
            ps_v0 = kvps.tile([P, 512], dt.float32, tag="qkv", name="kvps_t")
            for ci in range(NCC):
                nc.tensor.matmul(
                    ps_v0, lhsT=wkv[ci][:, P:2 * P], rhs=xt[ci][:, 0:512],
                    start=(ci == 0), stop=(ci == NCC - 1),
                )
            nc.scalar.copy(vraw[:, 0:512], ps_v0)

        # ---------------- Phase 1b: q projection, fused RoPE ---------------
        # Separate pool lifetime: phase 2's scores/y pools land on kvps's
        # banks (released long before) instead of inheriting this pool's
        # final drain.
        with tc.tile_pool(name="qps", bufs=3, space="PSUM") as qps:
            def q_chain(f, t4):
                ps = qps.tile([P, 512], dt.float32, tag="q", name="qps_t")
                for ci in range(NCC):
                    nc.tensor.matmul(
                        ps,
                        lhsT=wq[ci][:, f * P:(f + 1) * P],
                        rhs=xt[ci][:, t4 * 512:(t4 + 1) * 512],
                        start=(ci == 0), stop=(ci == NCC - 1),
                    )
                rope_drain(qrot[f], t4, ps)

            # vtps scope closes after f=1 so phase 2's y-PSUM pool (which
            # reuses these banks) only inherits the early transpose copies,
            # not the whole q-phase drain tail.
            with tc.tile_pool(name="vtps", bufs=2, space="PSUM") as vtps:
                for f in (0, 1):
                    for t4 in (3, 2, 1, 0):
                        q_chain(f, t4)
                        # v^T -> v (t-major [j-part, d]) via PE transpose,
                        # 2 per chain (bufs=2 so the pair never stalls on a
                        # copy); copies drain on ACT while chains stream.
                        base = (4 * f + (3 - t4)) * 2
                        for tb in (base, base + 1):
                            pst = vtps.tile([P, P], dt.bfloat16, tag="vtp", name="vtpst")
                            nc.tensor.transpose(pst, vraw[:, tb * P:(tb + 1) * P], identity)
                            nc.scalar.copy(vt[:, tb * P:(tb + 1) * P], pst)
            for f in (2, 3):
                for t4 in (3, 2, 1, 0):
                    q_chain(f, t4)

        # ------------- Phase 2: attention + interleaved partial proj --------
        # Quarter-major over 512-wide i-windows (largest first); scores^T
        # chunks [j-part, i-free], ACT exp, y^T and row sums via PE. The
        # PREVIOUS quarter's output projection is dripped into the j-block
        # loop so the PE always has work while ACT streams exps.
        with tc.tile_pool(name="spsp", bufs=1, space="PSUM") as spsp, \
             tc.tile_pool(name="prps", bufs=2, space="PSUM") as prps, \
             tc.tile_pool(name="ypsp", bufs=2, space="PSUM") as ypsp, \
             tc.tile_pool(name="scps", bufs=3, space="PSUM") as scps:

            def make_proj_items(q):
                # flattened op list: per (tb, ob): 4 chained matmuls then
                # copy+DMA. PSUM tile allocated lazily at emit time.
                items = []
                for tb in range(4 * q, 4 * q + 4):
                    t_sl = slice(tb * P, (tb + 1) * P)
                    for ob in range(4):
                        o_sl = slice(ob * 512, (ob + 1) * 512)
                        state = {}

                        def mk_mm(f4, t_sl=t_sl, o_sl=o_sl, state=state):
                            def _op():
                                if f4 == 0:
                                    state["pp"] = prps.tile(
                                        [P, 512], dt.float32, tag="pr", name="prpst")
                                nc.tensor.matmul(
                                    state["pp"],
                                    lhsT=y_sb[f4][:, t_sl],
                                    rhs=wp_t[f4][:, o_sl],
                                    start=(f4 == 0), stop=(f4 == NQ - 1),
                                )
                            return _op

                        def mk_fin(t_sl=t_sl, o_sl=o_sl, state=state):
                            def _op():
                                ot = ostage.tile([P, 512], dt.bfloat16,
                                                 tag="o", name="otile")
                                nc.vector.tensor_copy(ot, state["pp"])
                                nc.sync.dma_start(out_d[t_sl, o_sl], ot)
                            return _op

                        for f4 in range(NQ):
                            items.append(mk_mm(f4))
                        items.append(mk_fin())
                return items

            # Quarter order (1, 3, 2, 0): quarters are independent row-block
            # softmaxes, and each quarter's proj drips into the NEXT
            # quarter's attention loop - the un-dripped first quarter is
            # small, and its first row-sum matmul lands after the q-phase
            # drain tail clears the spsp bank.
            proj_items = []
            for q in (1, 3, 2, 0):
                q_lo = q * 512
                total_jbs = NQ * (4 * q + 4)
                rate = len(proj_items) / total_jbs if total_jbs else 0.0
                budget = 0.0
                for h in range(NQ):
                    qT = qrot[h]
                    yps = ypsp.tile([P, 512], dt.float32, tag="y", name="ypst")
                    sps = spsp.tile([P, 512], dt.float32, tag="s", name="spst")
                    njb = 4 * q + 4
                    sps_n = 0          # issued sps matmuls
                    sps_total = q + 4  # group sums + 4 diagonal pieces
                    g_acc = None       # running group accumulator
                    pending = None     # (acc_tile, ready_jb) for delayed ones-mm
                    d_hold = None      # held diagonal strip (pair grouping)
                    pend_diag = []     # deferred diagonal ones-mm pieces
                    tail_mm = None     # final diagonal piece, emitted post-drip
                    for jb in range(njb):
                        if pending is not None and jb >= pending[1]:
                            acc, _ = pending
                            nc.tensor.matmul(
                                sps, lhsT=ones_bf, rhs=acc,
                                start=(sps_n == 0), stop=(sps_n == sps_total - 1),
                            )
                            sps_n += 1
                            pending = None
                        j_sl = slice(jb * P, (jb + 1) * P)
                        i_lo = max(jb * P, q_lo)
                        w = q_lo + 512 - i_lo
                        c0 = 512 - w  # column offset inside the 512 window
                        strip = strip_pool.tile([P, 512], dt.bfloat16, tag="strip", name="stript")
                        ps = scps.tile([P, 512], dt.float32, tag="sc", name="scpst")
                        if jb >= 4 * q:
                            # diagonal block: preload causal mask into PSUM,
                            # then accumulate scores on top (first 128 cols);
                            # remaining cols are a fresh accumulation group.
                            nc.tensor.matmul(
                                ps[:, :P], lhsT=mskT, rhs=identity,
                                start=True, stop=False,
                            )
                            nc.tensor.matmul(
                                ps[:, :P], lhsT=krot[:, j_sl],
                                rhs=qT[:, i_lo:i_lo + P],
                                start=False, stop=True,
                            )
                            if w > P:
                                nc.tensor.matmul(
                                    ps[:, P:w], lhsT=krot[:, j_sl],
                                    rhs=qT[:, i_lo + P:i_lo + w],
                                    start=True, stop=True,
                                )
                        else:
                            nc.tensor.matmul(
                                ps[:, :w], lhsT=krot[:, j_sl],
                                rhs=qT[:, i_lo:i_lo + w],
                                start=True, stop=True,
                            )
                        nc.scalar.activation(strip[:, :w], ps[:, :w], AF.Exp, scale=SCALE)
                        nc.tensor.matmul(
                            yps[:, c0:], lhsT=vt[:, j_sl], rhs=strip[:, :w],
                            start=(jb == 0), stop=(jb == njb - 1),
                        )
                        if jb < 4 * q:
                            # full strip: sequentially accumulate groups of 4
                            # (adds 1,2 on DVE, final on GpSimd); ones-matmul
                            # per group issued 4 blocks later (clamped to the
                            # final iteration).
                            r = jb % 4
                            if r == 0:
                                g_acc = strip
                            else:
                                eng = nc.gpsimd if r == 3 else nc.vector
                                acc = gacc_pool.tile([P, 512], dt.bfloat16,
                                                     tag=f"ga{r}", name="gacct")
                                eng.tensor_add(acc, g_acc, strip)
                                g_acc = acc
                                if r == 3:
                                    pending = (g_acc, min(jb + 4, njb - 1))
                        else:
                            # diagonal strips pair-group: strips are stored
                            # left-aligned, so the overlap of consecutive
                            # widths (512/384 and 256/128) sums exactly with
                            # one offset DVE add each - the ones-matmul then
                            # streams 768 cols instead of 1280 per head.
                            db = jb - 4 * q
                            if db in (0, 2):
                                if db == 2:
                                    for src, off, wd in pend_diag:
                                        nc.tensor.matmul(
                                            sps[:, off:off + wd], lhsT=ones_bf,
                                            rhs=src[:, :wd],
                                            start=(sps_n == 0),
                                            stop=(sps_n == sps_total - 1),
                                        )
                                        sps_n += 1
                                    pend_diag = []
                                d_hold = strip
                            elif db == 1:
                                acc1 = strip_pool.tile([P, 512], dt.bfloat16,
                                                       tag="strip", name="stript")
                                nc.vector.tensor_add(
                                    acc1[:, 0:384], d_hold[:, 128:512], strip[:, 0:384])
                                pend_diag = [(d_hold, 0, 128), (acc1, 128, 384)]
                            else:  # db == 3
                                acc2 = strip_pool.tile([P, 512], dt.bfloat16,
                                                       tag="strip", name="stript")
                                nc.vector.tensor_add(
                                    acc2[:, 0:128], d_hold[:, 128:256], strip[:, 0:128])
                                nc.tensor.matmul(
                                    sps[:, 256:384], lhsT=ones_bf,
                                    rhs=d_hold[:, 0:128],
                                    start=(sps_n == 0), stop=(sps_n == sps_total - 1),
                                )
                                sps_n += 1
                                tail_mm = (acc2, 384, 128)
                        # drip the previous quarter's proj into this loop
                        budget += rate
                        while budget >= 1.0 and proj_items:
                            proj_items.pop(0)()
                            budget -= 1.0
                        if tail_mm is not None:
                            src, off, wd = tail_mm
                            tail_mm = None
                            nc.tensor.matmul(
                                sps[:, off:off + wd], lhsT=ones_bf, rhs=src[:, :wd],
                                start=(sps_n == 0), stop=(sps_n == sps_total - 1),
                            )
                            sps_n += 1
                    assert pending is None and sps_n == sps_total
                    # normalize: y * (1/rowsum) (sums broadcast on all partitions)
                    rcp = ssb_pool.tile([P, 512], dt.float32, tag="ssb", name="rcpt")
                    nc.vector.reciprocal_approx_fast(out=rcp, in_=sps)
                    nc.vector.tensor_mul(y_sb[h][:, q_lo:q_lo + 512], yps, rcp)
                # flush any leftover proj ops, then stage this quarter's
                for op in proj_items:
                    op()
                proj_items = make_proj_items(q)
            # tail: the last quarter's proj runs dense
            for op in proj_items:
                op()

    nc.compile()
    return nc


def kernel(x, w_attn, w_proj, cos, sin):
    x = np.asarray(x, dtype=np.float32)
    w_attn = np.asarray(w_attn, dtype=np.float32)
    w_proj = np.asarray(w_proj, dtype=np.float32)
    cos = np.asarray(cos, dtype=np.float32)
    sin = np.asarray(sin, dtype=np.float32)

    if "nc" not in _CACHE:
        _CACHE["nc"] = _build()
    nc = _CACHE["nc"]

    cosT = np.ascontiguousarray(cos.T).astype(BF16)         # [128, T] bf16
    sinT = np.ascontiguousarray(sin.T)
    sinS = sinT.copy()
    sinS[:64] = -sinS[:64]
    sinS = sinS.astype(BF16)

    in_maps = []
    for core in range(8):
        b, g = core // 4, core % 4
        wg = w_attn[g * 768:(g + 1) * 768]
        xT = np.ascontiguousarray(x[b].T).astype(BF16)                     # [C, T]
        wqT = np.ascontiguousarray(wg[:FQ].T).astype(BF16)                 # [C, 512]
        wkvT = np.ascontiguousarray(wg[FQ:].T).astype(BF16)                # [C, 256]
        wpT = np.ascontiguousarray(w_proj[:, g * FY:(g + 1) * FY].T).astype(BF16)  # [512, T]
        in_maps.append({"xT": xT, "wqT": wqT, "wkvT": wkvT, "wpT": wpT,
                        "cosT": cosT, "sinS": sinS})

    res = run_bass_kernel_spmd(nc, in_maps, core_ids=list(range(8)), trace=TRACE)
    if TRACE:
        _CACHE["last_results"] = res

    out = np.zeros((2, T, C), dtype=np.float32)
    for core in range(8):
        b = core // 4
        out[b] += res.results[core]["out"].astype(np.float32)
    return out


# revision 47
# speedup vs baseline: 1.1704x; 1.1704x over previous
"""Causal GQA self-attention (B=2, T=2048, C=2048, 16 heads / 4 KV groups,
head_size=128, RoPE) on 8 Trainium2 NeuronCores.

Sharding: tensor-parallel over the 4 KV groups x data-parallel over the 2
batch elements -> 8 cores, core = b*4 + g. Each core computes its group's
QKV projection, RoPE, causal SDPA for the group's 4 query heads, and the
partial output projection (w_proj input-dim shard). The proj partials are
reduced on the host (equivalent of the post-proj all-reduce).

All matmuls run in bf16 with fp32 PSUM accumulation. Inputs are transposed
and cast to bf16 on the host so every DMA is a contiguous, layout-perfect
load (contraction dims land on SBUF partitions).

Schedule notes (v7):
- Warmup junk matmuls (junk memset on DVE, first thing) + a dummy Exp run
  while the first DMAs land: PE p-state ramp + ACT exp-table load happen
  during dead startup time.
- w_attn is split host-side into wkv [C,256] and wq [C,512] chunk
  streams; the k+v projection runs 7 accumulation chains interleaved
  ci-outer so the PE consumes each (x, wkv) chunk pair at the DMA
  delivery rate. cos/sin land before the RoPE drains; wq streams in
  while the v-t4=0 chain and the first q chain run.
- Separate kvps/qps PSUM pool lifetimes: pool-level released-zone deps
  mean a new pool waits for ALL drains of any overlapped released pool,
  so attention's scores/y tiles are placed on banks whose pools released
  tens of microseconds earlier; the ones-matmul/proj pools tolerate the
  q-phase drain tail.
- v^T -> v t-major PE transposes ride one-per-q-chain (tb ascending);
  copies drain on ACT.
- The causal mask for diagonal blocks is preloaded into PSUM with a
  mask-base matmul and the scores matmul accumulates on top in two
  column pieces - no DVE hop in the scores->exp chain.
- Row sums: full-width exp strips accumulate sequentially in groups of 4
  (adds 1,2 on DVE, final on GpSimd), one ones-matmul per group issued 4
  blocks later; diagonal strips go straight to the ones-matmul.
- The partial output projection of quarter q is interleaved matmul-by-
  matmul into quarter q-1's attention loop: the ACT exp stream (~750ns
  per 512-wide strip) otherwise paces the PE (~650ns per j-block), so
  giving the PE extra independent work removes the exp-pacing bubbles.
  PSUM->SBUF copies ride DVE in bf16; bf16 output DMA (host upcasts and
  reduces in fp32).
"""

import sys
import math

for _p in ("/opt/trn_rl_repo", "/root/.axon_site/_ro/trn_rl_repo"):
    if _p not in sys.path:
        sys.path.insert(0, _p)

import numpy as np
import ml_dtypes

import concourse.bass as bass  # noqa: F401  (registers engine classes)
import concourse.bacc as bacc
import concourse.tile as tile
from concourse import mybir
from concourse.bass_utils import run_bass_kernel_spmd
from concourse.masks import make_identity
from contextlib import ExitStack

BF16 = ml_dtypes.bfloat16
P = 128
T = 2048
C = 2048
NT = T // P        # 16 t-blocks
NCC = C // P       # 16 contraction chunks
NQ = 4             # query heads per core
FQ = NQ * P        # 512 (q rows per group)
FKV = 2 * P        # 256 (k+v rows per group)
FY = NQ * P        # 512
SCALE = 1.0 / math.sqrt(P)
NEG = -1.0e30

dt = mybir.dt
AF = mybir.ActivationFunctionType
ALU = mybir.AluOpType

TRACE = False
_CACHE = {}


def _build():
    nc = bacc.Bacc("TRN2", target_bir_lowering=False, debug=False, num_devices=8)
    xT_d = nc.dram_tensor("xT", [C, T], dt.bfloat16, kind="ExternalInput").ap()
    wqT_d = nc.dram_tensor("wqT", [C, FQ], dt.bfloat16, kind="ExternalInput").ap()
    wkvT_d = nc.dram_tensor("wkvT", [C, FKV], dt.bfloat16, kind="ExternalInput").ap()
    wpT_d = nc.dram_tensor("wpT", [FY, T], dt.bfloat16, kind="ExternalInput").ap()
    cosT_d = nc.dram_tensor("cosT", [P, T], dt.bfloat16, kind="ExternalInput").ap()
    sinS_d = nc.dram_tensor("sinS", [P, T], dt.bfloat16, kind="ExternalInput").ap()
    out_d = nc.dram_tensor("out", [T, C], dt.bfloat16, kind="ExternalOutput").ap()

    with tile.TileContext(nc) as tc, ExitStack() as ctx:
        const = ctx.enter_context(tc.tile_pool(name="const", bufs=1))
        # junk tile memset FIRST (on DVE, which idles at startup) so PE
        # warmup can begin as soon as the engine queues come up.
        junk = const.tile([P, 512], dt.bfloat16, tag="junk", name="junk")
        nc.vector.memset(junk, 0.125)
        junk_exp = const.tile([P, 1], dt.float32, tag="jexp", name="junk_exp")
        identity = const.tile([P, P], dt.bfloat16, tag="id", name="identity")
        make_identity(nc, identity)
        ones_bf = const.tile([P, P], dt.bfloat16, tag="ones", name="ones_bf")
        nc.gpsimd.memset(ones_bf, 1.0)
        # transposed causal mask for the diagonal 128x128 block: loaded into
        # PSUM via matmul (lhsT=mskT, rhs=identity) so the PSUM base is
        # mskT.T: element (p=j, f=i) = 0 where i - j >= 0, else -1e30.
        mskT = const.tile([P, P], dt.bfloat16, tag="mskT", name="mskT")
        nc.gpsimd.memset(mskT, 0.0)
        nc.gpsimd.affine_select(
            out=mskT, in_=mskT, compare_op=ALU.is_ge, fill=NEG,
            base=0, pattern=[[-1, P]], channel_multiplier=1,
        )

        trig = ctx.enter_context(tc.tile_pool(name="trig", bufs=1))
        cosT = trig.tile([P, T], dt.bfloat16, tag="cos", name="cosT")
        sinS = trig.tile([P, T], dt.bfloat16, tag="sin", name="sinS")

        persist = ctx.enter_context(tc.tile_pool(name="persist", bufs=1))
        qrot = [persist.tile([P, T], dt.bfloat16, tag=f"q{h}", name=f"q{h}") for h in range(NQ)]
        krot = persist.tile([P, T], dt.bfloat16, tag="k", name="krot")
        vraw = persist.tile([P, T], dt.bfloat16, tag="vr", name="vraw")   # v^T (d-major)
        vt = persist.tile([P, T], dt.bfloat16, tag="vt", name="vt")       # v t-major blocks
        y_sb = [persist.tile([P, T], dt.bfloat16, tag=f"y{h}", name=f"ysb{h}") for h in range(NQ)]
        wp_t = [persist.tile([P, T], dt.bfloat16, tag=f"wp{j}", name=f"wp{j}") for j in range(NQ)]

        # DMA order: (x, wkv) chunk pairs stream first and pace the k+v
        # phase; cos/sin slot in before the RoPE drains need them; the wq
        # chunks follow (consumed from the first q chain onward), then wp.
        xw_pool = ctx.enter_context(tc.tile_pool(name="xw", bufs=1))
        xt, wkv, wq = [], [], []
        for ci in range(NCC):
            xt.append(xw_pool.tile([P, T], dt.bfloat16, tag=f"x{ci}", name=f"xt{ci}"))
            wkv.append(xw_pool.tile([P, FKV], dt.bfloat16, tag=f"wkv{ci}", name=f"wkv{ci}"))
            wq.append(xw_pool.tile([P, FQ], dt.bfloat16, tag=f"wq{ci}", name=f"wq{ci}"))

        def load_chunk(ci):
            nc.sync.dma_start(xt[ci], xT_d[ci * P:(ci + 1) * P, :])
            nc.sync.dma_start(wkv[ci], wkvT_d[ci * P:(ci + 1) * P, :])

        def load_wq(ci):
            nc.sync.dma_start(wq[ci], wqT_d[ci * P:(ci + 1) * P, :])

        # (x, wkv) pairs pace the k+v phase; cos/sin slot in before the
        # RoPE drains; wq follows the chunk stream (the v-t4=0 chain pads
        # the boundary, and the first q chain consumes wq gradually).
        for ci in range(0, 9):
            load_chunk(ci)
        nc.sync.dma_start(cosT, cosT_d)
        nc.sync.dma_start(sinS, sinS_d)
        for ci in range(9, NCC):
            load_chunk(ci)
        for ci in range(NCC):
            load_wq(ci)
        for j in range(NQ):
            nc.sync.dma_start(wp_t[j], wpT_d[j * P:(j + 1) * P, :])

        # Phase-2 SBUF pools allocated BEFORE rtmp (lower addresses) so the
        # attention phase never inherits rtmp's released-zone drain deps.
        strip_pool = ctx.enter_context(tc.tile_pool(name="strip", bufs=8))
        ssb_pool = ctx.enter_context(tc.tile_pool(name="ssb", bufs=3))
        ostage = ctx.enter_context(tc.tile_pool(name="ostage", bufs=4))
        gacc_pool = ctx.enter_context(tc.tile_pool(name="gacc", bufs=2))

        # ---------------- Phase 0: warmup ----------------------------------
        # PE p-state ramps to full clock after ~3us of continuous work; burn
        # the DMA-startup dead time on junk matmuls. Also trigger the ACT
        # exp table load now (1.3us) instead of at the first real softmax.
        nc.scalar.activation(junk_exp, junk[:, 0:1], AF.Exp, scale=1.0)
        with tc.tile_pool(name="warmps", bufs=2, space="PSUM") as warmps:
            for _ in range(9):
                wm_ps = warmps.tile([P, 512], dt.float32, tag="warm", name="warm_ps")
                nc.tensor.matmul(wm_ps, lhsT=junk[:, 0:P], rhs=junk,
                                 start=True, stop=True)

        # ---------------- Phase 1a: k/v projection, fused RoPE on k --------
        rtmp = ctx.enter_context(tc.tile_pool(name="rtmp", bufs=4))

        def rope_drain(dest, t4, ps, rot_eng=None):
            # RoPE (rotate-halves) in fp32, write bf16. The k drains pass
            # rot_eng=nc.gpsimd: all 4 kv chains stop only after the last
            # input chunk lands, so their rotate-muls would otherwise queue
            # ahead of the q-phase ropes on DVE and stall qps recycling.
            rot = rot_eng or nc.vector
            st = slice(t4 * 512, (t4 + 1) * 512)
            t1 = rtmp.tile([P, 512], dt.float32, tag="r1", name="ropet1")
            nc.vector.tensor_mul(t1, ps, cosT[:, st])
            t2 = rtmp.tile([P, 512], dt.float32, tag="r2", name="ropet2")
            rot.tensor_mul(t2[0:64, :], ps[64:128, :], sinS[0:64, st])
            rot.tensor_mul(t2[64:128, :], ps[0:64, :], sinS[64:128, st])
            nc.gpsimd.tensor_add(dest[:, st], t1, t2)

        with tc.tile_pool(name="kvps", bufs=7, space="PSUM") as kvps:
            # 7 accumulation chains interleaved ci-outer (k: t4=3..0,
            # v: t4=3..1) so the PE consumes chunks at the DMA rate;
            # v t4=0 follows as a single chain while the k drains run.
            kv_chains = [(0, 3), (0, 2), (0, 1), (0, 0), (1, 3), (1, 2), (1, 1)]
            pss = {c: kvps.tile([P, 512], dt.float32, tag="qkv", name="kvps_t")
                   for c in kv_chains}
            for ci in range(NCC):
                for fkv, t4 in kv_chains:
                    nc.tensor.matmul(
                        pss[(fkv, t4)],
                        lhsT=wkv[ci][:, fkv * P:(fkv + 1) * P],
                        rhs=xt[ci][:, t4 * 512:(t4 + 1) * 512],
                        start=(ci == 0), stop=(ci == NCC - 1),
                    )
            for t4 in (3, 2, 1, 0):
                rope_drain(krot, t4, pss[(0, t4)])
                if t4 > 0:
                    nc.scalar.copy(vraw[:, t4 * 512:(t4 + 1) * 512], pss[(1, t4)])
            ps_v0 = kvps.tile([P, 512], dt.float32, tag="qkv", name="kvps_t")
            for ci in range(NCC):
                nc.tensor.matmul(
                    ps_v0, lhsT=wkv[ci][:, P:2 * P], rhs=xt[ci][:, 0:512],
                    start=(ci == 0), stop=(ci == NCC - 1),
                )
            nc.scalar.copy(vraw[:, 0:512], ps_v0)

        # ---------------- Phase 1b: q projection, fused RoPE ---------------
        # Separate pool lifetime: phase 2's scores/y pools land on kvps's
        # banks (released long before) instead of inheriting this pool's
        # final drain.
        with tc.tile_pool(name="qps", bufs=3, space="PSUM") as qps:
            def q_chain(f, t4):
                ps = qps.tile([P, 512], dt.float32, tag="q", name="qps_t")
                for ci in range(NCC):
                    nc.tensor.matmul(
                        ps,
                        lhsT=wq[ci][:, f * P:(f + 1) * P],
                        rhs=xt[ci][:, t4 * 512:(t4 + 1) * 512],
                        start=(ci == 0), stop=(ci == NCC - 1),
                    )
                rope_drain(qrot[f], t4, ps)

            # vtps scope closes after f=1 so phase 2's y-PSUM pool (which
            # reuses these banks) only inherits the early transpose copies,
            # not the whole q-phase drain tail.
            with tc.tile_pool(name="vtps", bufs=2, space="PSUM") as vtps:
                for f in (0, 1):
                    for t4 in (3, 2, 1, 0):
                        q_chain(f, t4)
                        # v^T -> v (t-major [j-part, d]) via PE transpose,
                        # 2 per chain (bufs=2 so the pair never stalls on a
                        # copy); copies drain on ACT while chains stream.
                        base = (4 * f + (3 - t4)) * 2
                        for tb in (base, base + 1):
                            pst = vtps.tile([P, P], dt.bfloat16, tag="vtp", name="vtpst")
                            nc.tensor.transpose(pst, vraw[:, tb * P:(tb + 1) * P], identity)
                            nc.scalar.copy(vt[:, tb * P:(tb + 1) * P], pst)
            for f in (2, 3):
                for t4 in (3, 2, 1, 0):
                    q_chain(f, t4)

        # ------------- Phase 2: attention + interleaved partial proj --------
        # Quarter-major over 512-wide i-windows (largest first); scores^T
        # chunks [j-part, i-free], ACT exp, y^T and row sums via PE. The
        # PREVIOUS quarter's output projection is dripped into the j-block
        # loop so the PE always has work while ACT streams exps.
        with tc.tile_pool(name="spsp", bufs=1, space="PSUM") as spsp, \
             tc.tile_pool(name="prps", bufs=2, space="PSUM") as prps, \
             tc.tile_pool(name="ypsp", bufs=2, space="PSUM") as ypsp, \
             tc.tile_pool(name="scps", bufs=3, space="PSUM") as scps:

            def make_proj_items(q):
                # flattened op list: per (tb, ob): 4 chained matmuls then
                # copy+DMA. PSUM tile allocated lazily at emit time.
                items = []
                for tb in range(4 * q, 4 * q + 4):
                    t_sl = slice(tb * P, (tb + 1) * P)
                    for ob in range(4):
                        o_sl = slice(ob * 512, (ob + 1) * 512)
                        state = {}

                        def mk_mm(f4, t_sl=t_sl, o_sl=o_sl, state=state):
                            def _op():
                                if f4 == 0:
                                    state["pp"] = prps.tile(
                                        [P, 512], dt.float32, tag="pr", name="prpst")
                                nc.tensor.matmul(
                                    state["pp"],
                                    lhsT=y_sb[f4][:, t_sl],
                                    rhs=wp_t[f4][:, o_sl],
                                    start=(f4 == 0), stop=(f4 == NQ - 1),
                                )
                            return _op

                        def mk_fin(t_sl=t_sl, o_sl=o_sl, state=state):
                            def _op():
                                ot = ostage.tile([P, 512], dt.bfloat16,
                                                 tag="o", name="otile")
                                nc.vector.tensor_copy(ot, state["pp"])
                                nc.sync.dma_start(out_d[t_sl, o_sl], ot)
                            return _op

                        for f4 in range(NQ):
                            items.append(mk_mm(f4))
                        items.append(mk_fin())
                return items

            # Quarter order (1, 3, 2, 0): quarters are independent row-block
            # softmaxes, and each quarter's proj drips into the NEXT
            # quarter's attention loop - the un-dripped first quarter is
            # small, and its first row-sum matmul lands after the q-phase
            # drain tail clears the spsp bank.
            proj_items = []
            for q in (1, 3, 2, 0):
                q_lo = q * 512
                total_jbs = NQ * (4 * q + 4)
                rate = len(proj_items) / total_jbs if total_jbs else 0.0
                budget = 0.0
                for h in range(NQ):
                    qT = qrot[h]
                    yps = ypsp.tile([P, 512], dt.float32, tag="y", name="ypst")
                    sps = spsp.tile([P, 512], dt.float32, tag="s", name="spst")
                    njb = 4 * q + 4
                    sps_n = 0          # issued sps matmuls
                    sps_total = q + 4  # group sums + 4 diagonal pieces
                    g_acc = None       # running group accumulator
                    pending = None     # (acc_tile, ready_jb) for delayed ones-mm
                    d_hold = None      # held diagonal strip (pair grouping)
                    pend_diag = []     # deferred diagonal ones-mm pieces
                    tail_mm = None     # final diagonal piece, emitted post-drip
                    for jb in range(njb):
                        if pending is not None and jb >= pending[1]:
                            acc, _ = pending
                            nc.tensor.matmul(
                                sps, lhsT=ones_bf, rhs=acc,
                                start=(sps_n == 0), stop=(sps_n == sps_total - 1),
                            )
                            sps_n += 1
                            pending = None
                        j_sl = slice(jb * P, (jb + 1) * P)
                        i_lo = max(jb * P, q_lo)
                        w = q_lo + 512 - i_lo
                        c0 = 512 - w  # column offset inside the 512 window
                        strip = strip_pool.tile([P, 512], dt.bfloat16, tag="strip", name="stript")
                        ps = scps.tile([P, 512], dt.float32, tag="sc", name="scpst")
                        if jb >= 4 * q:
                            # diagonal block: preload causal mask into PSUM,
                            # then accumulate scores on top (first 128 cols);
                            # remaining cols are a fresh accumulation group.
                            nc.tensor.matmul(
                                ps[:, :P], lhsT=mskT, rhs=identity,
                                start=True, stop=False,
                            )
                            nc.tensor.matmul(
                                ps[:, :P], lhsT=krot[:, j_sl],
                                rhs=qT[:, i_lo:i_lo + P],
                                start=False, stop=True,
                            )
                            if w > P:
                                nc.tensor.matmul(
                                    ps[:, P:w], lhsT=krot[:, j_sl],
                                    rhs=qT[:, i_lo + P:i_lo + w],
                                    start=True, stop=True,
                                )
                        else:
                            nc.tensor.matmul(
                                ps[:, :w], lhsT=krot[:, j_sl],
                                rhs=qT[:, i_lo:i_lo + w],
                                start=True, stop=True,
                            )
                        nc.scalar.activation(strip[:, :w], ps[:, :w], AF.Exp, scale=SCALE)
                        nc.tensor.matmul(
                            yps[:, c0:], lhsT=vt[:, j_sl], rhs=strip[:, :w],
                            start=(jb == 0), stop=(jb == njb - 1),
                        )
                        if jb < 4 * q:
                            # full strip: sequentially accumulate groups of 4
                            # (adds 1,2 on DVE, final on GpSimd); ones-matmul
                            # per group issued 4 blocks later (clamped to the
                            # final iteration).
                            r = jb % 4
                            if r == 0:
                                g_acc = strip
                            else:
                                eng = nc.gpsimd if r == 3 else nc.vector
                                acc = gacc_pool.tile([P, 512], dt.bfloat16,
                                                     tag=f"ga{r}", name="gacct")
                                eng.tensor_add(acc, g_acc, strip)
                                g_acc = acc
                                if r == 3:
                                    pending = (g_acc, min(jb + 4, njb - 1))
                        else:
                            # diagonal strips pair-group: strips are stored
                            # left-aligned, so the overlap of consecutive
                            # widths (512/384 and 256/128) sums exactly with
                            # one offset DVE add each - the ones-matmul then
                            # streams 768 cols instead of 1280 per head.
                            db = jb - 4 * q
                            if db in (0, 2):
                                if db == 2:
                                    for src, off, wd in pend_diag:
                                        nc.tensor.matmul(
                                            sps[:, off:off + wd], lhsT=ones_bf,
                                            rhs=src[:, :wd],
                                            start=(sps_n == 0),
                                            stop=(sps_n == sps_total - 1),
                                        )
                                        sps_n += 1
                                    pend_diag = []
                                d_hold = strip
                            elif db == 1:
                                acc1 = strip_pool.tile([P, 512], dt.bfloat16,
                                                       tag="strip", name="stript")
                                nc.vector.tensor_add(
                                    acc1[:, 0:384], d_hold[:, 128:512], strip[:, 0:384])
                                pend_diag = [(d_hold, 0, 128), (acc1, 128, 384)]
                            else:  # db == 3
                                acc2 = strip_pool.tile([P, 512], dt.bfloat16,
                                                       tag="strip", name="stript")
                                nc.vector.tensor_add(
                                    acc2[:, 0:128], d_hold[:, 128:256], strip[:, 0:128])
                                nc.tensor.matmul(
                                    sps[:, 256:384], lhsT=ones_bf,
                                    rhs=d_hold[:, 0:128],
                                    start=(sps_n == 0), stop=(sps_n == sps_total - 1),
                                )
                                sps_n += 1
                                tail_mm = (acc2, 384, 128)
                        # drip the previous quarter's proj into this loop
                        budget += rate
                        while budget >= 1.0 and proj_items:
                            proj_items.pop(0)()
                            budget -= 1.0
                        if tail_mm is not None:
                            src, off, wd = tail_mm
                            tail_mm = None
                            nc.tensor.matmul(
                                sps[:, off:off + wd], lhsT=ones_bf, rhs=src[:, :wd],
                                start=(sps_n == 0), stop=(sps_n == sps_total - 1),
                            )
                            sps_n += 1
                    assert pending is None and sps_n == sps_total
                    # normalize: y * (1/rowsum) (sums broadcast on all partitions)
                    rcp = ssb_pool.tile([P, 512], dt.float32, tag="ssb", name="rcpt")
                    nc.vector.reciprocal_approx_fast(out=rcp, in_=sps)
                    nc.vector.tensor_mul(y_sb[h][:, q_lo:q_lo + 512], yps, rcp)
                # flush any leftover proj ops, then stage this quarter's
                for op in proj_items:
                    op()
                proj_items = make_proj_items(q)
            # tail: the last quarter's proj runs dense
            for op in proj_items:
                op()

    nc.compile()
    return nc


def kernel(x, w_attn, w_proj, cos, sin):
    x = np.asarray(x, dtype=np.float32)
    w_attn = np.asarray(w_attn, dtype=np.float32)
    w_proj = np.asarray(w_proj, dtype=np.float32)
    cos = np.asarray(cos, dtype=np.float32)
    sin = np.asarray(sin, dtype=np.float32)

    if "nc" not in _CACHE:
        _CACHE["nc"] = _build()
    nc = _CACHE["nc"]

    cosT = np.ascontiguousarray(cos.T).astype(BF16)         # [128, T] bf16
    sinT = np.ascontiguousarray(sin.T)
    sinS = sinT.copy()
    sinS[:64] = -sinS[:64]
    sinS = sinS.astype(BF16)

    in_maps = []
    for core in range(8):
        b, g = core // 4, core % 4
        wg = w_attn[g * 768:(g + 1) * 768]
        xT = np.ascontiguousarray(x[b].T).astype(BF16)                     # [C, T]
        wqT = np.ascontiguousarray(wg[:FQ].T).astype(BF16)                 # [C, 512]
        wkvT = np.ascontiguousarray(wg[FQ:].T).astype(BF16)                # [C, 256]
        wpT = np.ascontiguousarray(w_proj[:, g * FY:(g + 1) * FY].T).astype(BF16)  # [512, T]
        in_maps.append({"xT": xT, "wqT": wqT, "wkvT": wkvT, "wpT": wpT,
                        "cosT": cosT, "sinS": sinS})

    res = run_bass_kernel_spmd(nc, in_maps, core_ids=list(range(8)), trace=TRACE)
    if TRACE:
        _CACHE["last_results"] = res

    out = np.zeros((2, T, C), dtype=np.float32)
    for core in range(8):
        b = core // 4
        out[b] += res.results[core]["out"].astype(np.float32)
    return out
